# revision 1
# baseline (speedup 1.0000x reference)
"""Transformer block (QKV + causal MHA + proj + GELU-FF, residual) on 8 NeuronCores.

Sharding: DP over batch (2 groups of 4 cores) x TP over heads / FF-inner within
each group. Identical SPMD program on all cores; per-core differences are input
slices only. Activations are feature-major end to end; all matmul operands bf16
(PSUM accumulation fp32). xT is loaded once and kept resident in SBUF for P1a
(QK), P1b (V) and FF1. FF1 for token chunks 0-1 is interleaved into the
attention kt loop as PE filler (the PE queue is strict FIFO; filler matmuls
cover the ScalarE exp latency between score and pv matmuls); FF1 for chunks
2-3 runs after FF2 of chunks 0-1 frees their h buffers (emitting them earlier
would deadlock the PE FIFO on the h WAR). exp tiles accumulate on the DVE; one
ones-matmul per (h,c) group forms the softmax denominators. proj and ff2
partials share a PSUM group; token-chunked bf16 ReduceScatter per 4-core
group, with end-of-rep output copies on the gpsimd queue. Host adds x + b_ff2
(residual) during unshard.
"""
import numpy as np
import ml_dtypes

import concourse.bass as bass
import concourse.bass_isa as bass_isa
import concourse.mybir as mybir
import concourse.tile as tile
from concourse import bacc
from concourse import bass_utils

B, T, C = 2, 2048, 2048
H, HD = 16, 128
F = 8192
NCORES = 8
TPG = 4                  # cores per batch group
HPC = H // TPG           # heads per core
QC = 4                   # token chunks per batch
TCH = T // QC            # 512
KT = C // 128            # 16
FPC = F // TPG           # 2048 ff rows per core
FT = FPC // 128          # 16
COT = C // 128           # 16
SM_SCALE = 1.0 / float(np.sqrt(HD))
NEG = -60000.0

f32r = mybir.dt.float32r
f32 = mybir.dt.float32
bf16 = mybir.dt.bfloat16

_CACHED_NC = None


def build_nc(rep=1, do_p12=True, do_p3=True, rs_mode="chunked"):
    nc = bacc.Bacc("TRN2", target_bir_lowering=False, debug=False,
                   num_devices=NCORES)
    xT_t = nc.dram_tensor("xT", [C, T], bf16, kind="ExternalInput").ap()
    wqk_t = nc.dram_tensor("wqk", [C, 2 * HPC * HD], bf16, kind="ExternalInput").ap()
    wv_t = nc.dram_tensor("wv", [C, HPC * HD], bf16, kind="ExternalInput").ap()
    wp_t = nc.dram_tensor("wp", [HPC * HD, C], bf16, kind="ExternalInput").ap()
    w1_t = nc.dram_tensor("w1", [C, FPC], bf16, kind="ExternalInput").ap()
    b1_t = nc.dram_tensor("b1", [128, FT], f32, kind="ExternalInput").ap()
    w2_t = nc.dram_tensor("w2", [FPC, C], bf16, kind="ExternalInput").ap()
    out_t = nc.dram_tensor("outp", [C // TPG, T], bf16, kind="ExternalOutput").ap()

    xT_v = xT_t.rearrange("(kt p) t -> p kt t", p=128)
    wqk_v = wqk_t.rearrange("(kt p) f -> p kt f", p=128)
    wv_v = wv_t.rearrange("(kt p) f -> p kt f", p=128)
    wp_v = wp_t.rearrange("(kt p) c -> p kt c", p=128)
    w1_v = w1_t.rearrange("(kt p) f -> p kt f", p=128)
    w2_v = w2_t.rearrange("(ft p) c -> p ft c", p=128)

    with tile.TileContext(nc) as tc:
        with tc.tile_pool(name="cstp", bufs=1) as cst, \
             tc.tile_pool(name="ps", bufs=1, space="PSUM") as ps, \
             tc.tile_pool(name="dram", bufs=1, space="DRAM") as dram:

            masks = cst.tile([128, QC, TCH], f32, name="masks", tag="mask")
            nc.gpsimd.memset(masks[:], 0.0)
            for d in range(QC):
                nc.gpsimd.affine_select(
                    out=masks[:, d, :], in_=masks[:, d, :],
                    compare_op=mybir.AluOpType.is_ge,
                    fill=NEG, base=-d * 128,
                    pattern=[[1, TCH]], channel_multiplier=-1,
                )
            b1_sb = cst.tile([128, FT], f32, name="b1_sb", tag="b1")
            nc.sync.dma_start(b1_sb[:], b1_t)

            with tc.tile_pool(name="xp", bufs=1) as xp, \
                 tc.tile_pool(name="p1p", bufs=1) as p1p:
              xT_sb = xp.tile([128, KT, T], bf16, name="xT_sb", tag="xT")
              for c in range(QC):
                  nc.sync.dma_start(xT_sb[:, :, c * TCH:(c + 1) * TCH],
                                    xT_v[:, :, c * TCH:(c + 1) * TCH])
              for _rep in range(rep):
                with tc.tile_pool(name="attp", bufs=1) as attp, \
                     tc.tile_pool(name="ff1p", bufs=1) as ff1p:
                    attnT = attp.tile([128, HPC, T], bf16, name="attnT",
                                      tag="attnT")

                    # --- FF1 job-stream machinery -------------------------
                    ff1_jobs = [(c, f) for c in range(QC) for f in range(FT)]
                    ff1_dmas = {}
                    ff1_cur = [None]
                    ff1_ptr = [0]
                    h_tiles = {}

                    def h_tile(c):
                        if c not in h_tiles:
                            h_tiles[c] = ff1p.tile([128, FT, TCH], bf16,
                                                   name="h_sb", tag="h",
                                                   bufs=2)
                        return h_tiles[c]

                    def ff1_w1dma(job):
                        if job >= len(ff1_jobs) or job in ff1_dmas:
                            return
                        _, f = ff1_jobs[job]
                        t = ff1p.tile([128, KT, 128], bf16, name="w1tile",
                                      tag="w1t", bufs=2)
                        nc.sync.dma_start(t[:],
                                          w1_v[:, :, f * 128:(f + 1) * 128])
                        ff1_dmas[job] = t

                    def ff1_emit(n_mms, limit_jobs):
                        # limit_jobs caps which jobs may be emitted: a chunk's
                        # jobs are only legal once the FF2 that frees its h
                        # buffer slot has been emitted (else the h WAR
                        # deadlocks the scalar FIFO).
                        for _ in range(n_mms):
                            idx = ff1_ptr[0]
                            if idx >= limit_jobs * KT:
                                return
                            job, k = divmod(idx, KT)
                            c, f = ff1_jobs[job]
                            if k == 0:
                                ff1_w1dma(job)
                                if job + 1 < limit_jobs:
                                    ff1_w1dma(job + 1)
                                ph = ps.tile([128, TCH], f32, name="ph",
                                             tag="ph", bufs=3)
                                ff1_cur[0] = (ff1_dmas.pop(job), ph)
                            w1tile, ph = ff1_cur[0]
                            nc.tensor.matmul(
                                ph[:], w1tile[:, k, :],
                                xT_sb[:, k, c * TCH:(c + 1) * TCH],
                                start=(k == 0), stop=(k == KT - 1))
                            if k == KT - 1:
                                nc.scalar.activation(
                                    h_tile(c)[:, f, :], ph[:],
                                    mybir.ActivationFunctionType.Gelu,
                                    bias=b1_sb[:, f:f + 1], scale=1.0)
                            ff1_ptr[0] = idx + 1

                    # ------------------------------------------------------

                    if not do_p12:
                        nc.gpsimd.memset(attnT[:], 0.01)
                    if do_p12:
                      with tc.tile_pool(name="qkvp", bufs=1) as qkvp:
                        qk_sb = qkvp.tile([128, 2 * HPC, T], bf16,
                                          name="qk_sb", tag="qk")
                        v_sb = qkvp.tile([128, T // 128, HPC * HD], bf16,
                                         name="v_sb", tag="v")

                        with tc.tile_pool(name="p1w", bufs=1) as p1w:
                            # P1a: qT/kT = w_qk^T @ x (feature-major);
                            # wqk streamed per 128-col tile from the
                            # persistent pool so the next rep's tiles
                            # prefetch during this rep's FF2
                            for ft in range(2 * HPC):
                                wqkt = p1p.tile([128, KT, 128], bf16,
                                                name="wqkt", tag="wqkt",
                                                bufs=3)
                                nc.sync.dma_start(
                                    wqkt[:],
                                    wqk_v[:, :, ft * 128:(ft + 1) * 128])
                                for c in range(QC):
                                    pt = ps.tile([128, TCH], f32, name="pmm",
                                                 tag="pmm", bufs=3)
                                    for k in range(KT):
                                        nc.tensor.matmul(
                                            pt[:],
                                            wqkt[:, k, :],
                                            xT_sb[:, k, c * TCH:(c + 1) * TCH],
                                            start=(k == 0), stop=(k == KT - 1))
                                    nc.vector.tensor_copy(
                                        qk_sb[:, ft, c * TCH:(c + 1) * TCH],
                                        pt[:])

                            # P1b: v = x @ w_v (token-major)
                            wv_sb = p1w.tile([128, KT, HPC * HD], bf16,
                                             name="wv_sb", tag="wv", bufs=1)
                            nc.sync.dma_start(wv_sb[:], wv_v)
                            for m in range(T // 128):
                                pt = ps.tile([128, HPC * HD], f32, name="pmm",
                                             tag="pmm", bufs=3)
                                for k in range(KT):
                                    nc.tensor.matmul(
                                        pt[:],
                                        xT_sb[:, k, m * 128:(m + 1) * 128],
                                        wv_sb[:, k, :],
                                        start=(k == 0), stop=(k == KT - 1))
                                nc.vector.tensor_copy(v_sb[:, m, :], pt[:])

                        # P2: causal attention with FF1(c0,c1) interleaved
                        # as PE filler between score(kt+1) and pv(kt).
                        with tc.tile_pool(name="p2w", bufs=1) as p2w:
                            LA = 1

                            def emit_score_exp(h, c, kt):
                                # diagonal block d contributes only to query
                                # cols >= d*128
                                off = max(0, (kt - 4 * c) * 128)
                                pscore = ps.tile([128, TCH], f32, name="pmm",
                                                 tag="pmm", bufs=3)
                                nc.tensor.matmul(
                                    pscore[:, off:],
                                    qk_sb[:, HPC + h,
                                          kt * 128:(kt + 1) * 128],
                                    qk_sb[:, h,
                                          c * TCH + off:(c + 1) * TCH],
                                    start=True, stop=True)
                                e_sb = p2w.tile([128, TCH], bf16, name="e_sb",
                                                tag="e", bufs=3)
                                if kt >= 4 * c:
                                    d = kt - 4 * c
                                    ms = p2w.tile([128, TCH], f32, name="ms",
                                                  tag="ms", bufs=2)
                                    nc.vector.tensor_add(
                                        ms[:, off:], pscore[:, off:],
                                        masks[:, d, off:])
                                    nc.scalar.activation(
                                        e_sb[:, off:], ms[:, off:],
                                        mybir.ActivationFunctionType.Exp,
                                        scale=SM_SCALE)
                                else:
                                    nc.scalar.activation(
                                        e_sb[:], pscore[:],
                                        mybir.ActivationFunctionType.Exp,
                                        scale=SM_SCALE)
                                return e_sb, off

                            def emit_norm(st):
                                po, e_sum, h, c = st
                                rb = p2w.tile([128, TCH], f32, name="rb",
                                              tag="rb", bufs=2)
                                nc.vector.reciprocal(rb[:], e_sum[:])
                                nc.vector.tensor_mul(
                                    attnT[:, h, c * TCH:(c + 1) * TCH],
                                    po[:], rb[:])

                            pending = None
                            for h in range(HPC):
                                for c in range(QC):
                                    nkt = 4 * c + 4
                                    po = ps.tile([128, TCH], f32, name="po",
                                                 tag="po", bufs=2)
                                    e_acc = p2w.tile([128, TCH], f32,
                                                     name="e_acc", tag="eacc",
                                                     bufs=2)
                                    es = {}
                                    for kt in range(min(LA, nkt)):
                                        es[kt] = emit_score_exp(h, c, kt)
                                    for kt in range(nkt):
                                        if kt + LA < nkt:
                                            es[kt + LA] = emit_score_exp(
                                                h, c, kt + LA)
                                        ff1_emit(3, 2 * FT)
                                        e_sb, off = es.pop(kt)
                                        if kt == 0:
                                            nc.vector.tensor_copy(
                                                e_acc[:], e_sb[:])
                                        else:
                                            nc.vector.tensor_add(
                                                e_acc[:, off:],
                                                e_acc[:, off:],
                                                e_sb[:, off:])
                                        nc.tensor.matmul(
                                            po[:, off:],
                                            v_sb[:, kt, h * HD:(h + 1) * HD],
                                            e_sb[:, off:], start=(kt == 0),
                                            stop=(kt == nkt - 1))
                                    e_sum = p2w.tile([128, TCH], f32,
                                                     name="e_sum", tag="esum",
                                                     bufs=2)
                                    nc.gpsimd.partition_all_reduce(
                                        e_sum[:], e_acc[:], channels=128,
                                        reduce_op=bass_isa.ReduceOp.add)
                                    if pending is not None:
                                        # deferred a full group: the gpsimd
                                        # reduce gets a group-span of slack
                                        # before the DVE waits on it
                                        emit_norm(pending)
                                    pending = (po, e_sum, h, c)
                            emit_norm(pending)

                    if not do_p3:
                        nc.sync.dma_start(out_t[0:128, :], attnT[:, 0, :])
                    if do_p3:
                      with tc.tile_pool(name="p3w", bufs=1) as p3w:
                        ff1_emit(10 ** 9, 2 * FT)   # leftovers of chunks 0-1
                        wp_sb = p3w.tile([128, TPG, C], bf16, name="wp_sb",
                                         tag="wp", bufs=1)
                        nc.sync.dma_start(wp_sb[:], wp_v)
                        rs_out_all = dram.tile([QC, (COT * 128) // TPG, TCH],
                                               bf16, name="rs_out_all",
                                               tag="rsoa", bufs=2)

                        def emit_ff2(c, prefetch_jobs=0):
                            ht = h_tiles[c]
                            rs_in = dram.tile([COT * 128, TCH], bf16,
                                              name="rs_in", tag="rsi", bufs=2)
                            for co in range(COT):
                                if co == COT - 2 and prefetch_jobs:
                                    # warm the next FF1 chunk's first weight
                                    # tiles so its matmuls start without a
                                    # DMA cold-stall (PE p-state ramp)
                                    j = ff1_ptr[0] // KT
                                    ff1_w1dma(j)
                                    ff1_w1dma(j + 1)
                                w2tile = p3w.tile([128, FT, 128], bf16,
                                                  name="w2tile", tag="w2t",
                                                  bufs=3)
                                nc.sync.dma_start(
                                    w2tile[:],
                                    w2_v[:, :, co * 128:(co + 1) * 128])
                                pout = ps.tile([128, TCH], f32, name="pmm",
                                               tag="pmm", bufs=3)
                                # ff2 first, proj last: the first groups can
                                # start before wp finishes loading
                                for f in range(FT):
                                    nc.tensor.matmul(
                                        pout[:], w2tile[:, f, :], ht[:, f, :],
                                        start=(f == 0), stop=False)
                                for k4 in range(TPG):
                                    nc.tensor.matmul(
                                        pout[:],
                                        wp_sb[:, k4, co * 128:(co + 1) * 128],
                                        attnT[:, k4, c * TCH:(c + 1) * TCH],
                                        start=False, stop=(k4 == TPG - 1))
                                o_sb = p3w.tile([128, TCH], bf16, name="o_sb",
                                                tag="o", bufs=2)
                                nc.vector.tensor_copy(o_sb[:], pout[:])
                                # staging store on the scalar HWDGE queue
                                nc.scalar.dma_start(
                                    rs_in[co * 128:(co + 1) * 128, :], o_sb[:])
                            if rs_mode == "chunked":
                                nc.gpsimd.collective_compute(
                                    "ReduceScatter", mybir.AluOpType.add,
                                    replica_groups=[[0, 1, 2, 3], [4, 5, 6, 7]],
                                    ins=[rs_in.opt()], outs=[rs_out_all[c]])
                            elif rs_mode == "none":
                                nc.sync.dma_start(
                                    out_t[:, c * TCH:(c + 1) * TCH],
                                    rs_in[0:512, :])

                        if 0 not in h_tiles:        # do_p12=False ablation
                            ff1_emit(10 ** 9, 2 * FT)
                        emit_ff2(0, prefetch_jobs=1)
                        ff1_emit(10 ** 9, 3 * FT)   # chunk 2
                        emit_ff2(1, prefetch_jobs=1)
                        ff1_emit(10 ** 9, 4 * FT)   # chunk 3
                        emit_ff2(2)
                        emit_ff2(3)
                        if rs_mode == "chunked":
                            # end-of-rep out copies on the gpsimd queue
                            # (after all RS triggers)
                            for c in range(QC):
                                nc.gpsimd.dma_start(
                                    out_t[:, c * TCH:(c + 1) * TCH],
                                    rs_out_all[c])

    nc.compile()
    return nc


def make_in_maps(x, w_qkv, w_proj, w_ff1, b_ff1, w_ff2):
    in_maps = []
    asc = np.ascontiguousarray
    bf = ml_dtypes.bfloat16
    for r in range(NCORES):
        b, hg = r // TPG, r % TPG
        q_cols = w_qkv[:, hg * 512:(hg + 1) * 512]
        k_cols = w_qkv[:, C + hg * 512:C + (hg + 1) * 512]
        v_cols = w_qkv[:, 2 * C + hg * 512:2 * C + (hg + 1) * 512]
        xT = asc(x[b].T)
        in_maps.append({
            "xT": xT.astype(bf),
            "wqk": asc(np.concatenate([q_cols, k_cols], axis=1)).astype(bf),
            "wv": asc(v_cols).astype(bf),
            "wp": asc(w_proj[hg * 512:(hg + 1) * 512, :]).astype(bf),
            "w1": asc(w_ff1[:, hg * FPC:(hg + 1) * FPC]).astype(bf),
            "b1": asc(b_ff1[hg * FPC:(hg + 1) * FPC].reshape(FT, 128).T),
            "w2": asc(w_ff2[hg * FPC:(hg + 1) * FPC, :]).astype(bf),
        })
    return in_maps


def assemble(results, x, b_ff2):
    out = np.empty((B, T, C), np.float32)
    for r in range(NCORES):
        b, idx = r // TPG, r % TPG
        out[b, :, idx * 512:(idx + 1) * 512] = \
            results[r]["outp"].astype(np.float32).T
    out += x + b_ff2
    return out


def kernel(x, w_qkv, w_proj, w_ff1, b_ff1, w_ff2, b_ff2):
    global _CACHED_NC
    x = np.asarray(x, np.float32)
    if _CACHED_NC is None:
        _CACHED_NC = build_nc()
    in_maps = make_in_maps(x, np.asarray(w_qkv, np.float32),
                           np.asarray(w_proj, np.float32),
                           np.asarray(w_ff1, np.float32),
                           np.asarray(b_ff1, np.float32),
                           np.asarray(w_ff2, np.float32))
    res = bass_utils.run_bass_kernel_spmd(_CACHED_NC, in_maps,
                                          core_ids=list(range(NCORES)))
    return assemble(res.results, x, np.asarray(b_ff2, np.float32))



# revision 4
# speedup vs baseline: 1.0360x; 1.0360x over previous
"""Transformer block (QKV + causal MHA + proj + GELU-FF, residual) on 8 NeuronCores.

Sharding: DP over batch (2 groups of 4 cores) x TP over heads / FF-inner within
each group. Identical SPMD program on all cores; per-core differences are input
slices only. Activations are feature-major end to end; all matmul operands bf16
(PSUM accumulation fp32). xT is loaded once and kept resident in SBUF for P1a
(QK), P1b (V) and FF1. FF1 for token chunks 0-1 is interleaved into the
attention kt loop as PE filler (the PE queue is strict FIFO; filler matmuls
cover the ScalarE exp latency between score and pv matmuls); FF1 for chunks
2-3 runs after FF2 of chunks 0-1 frees their h buffers (emitting them earlier
would deadlock the PE FIFO on the h WAR). exp tiles accumulate on the DVE; one
ones-matmul per (h,c) group forms the softmax denominators. proj and ff2
partials share a PSUM group; token-chunked bf16 ReduceScatter per 4-core
group, with end-of-rep output copies on the gpsimd queue. Host adds x + b_ff2
(residual) during unshard.
"""
import numpy as np
import ml_dtypes

import concourse.bass as bass
import concourse.bass_isa as bass_isa
import concourse.mybir as mybir
import concourse.tile as tile
from concourse import bacc
from concourse import bass_utils

B, T, C = 2, 2048, 2048
H, HD = 16, 128
F = 8192
NCORES = 8
TPG = 4                  # cores per batch group
HPC = H // TPG           # heads per core
QC = 4                   # token chunks per batch
TCH = T // QC            # 512
KT = C // 128            # 16
FPC = F // TPG           # 2048 ff rows per core
FT = FPC // 128          # 16
COT = C // 128           # 16
SM_SCALE = 1.0 / float(np.sqrt(HD))
NEG = -60000.0

f32r = mybir.dt.float32r
f32 = mybir.dt.float32
bf16 = mybir.dt.bfloat16

_CACHED_NC = None


def build_nc(rep=1, do_p12=True, do_p3=True, rs_mode="chunked"):
    nc = bacc.Bacc("TRN2", target_bir_lowering=False, debug=False,
                   num_devices=NCORES)
    xT_t = nc.dram_tensor("xT", [C, T], bf16, kind="ExternalInput").ap()
    wqk_t = nc.dram_tensor("wqk", [C, 2 * HPC * HD], bf16, kind="ExternalInput").ap()
    wv_t = nc.dram_tensor("wv", [C, HPC * HD], bf16, kind="ExternalInput").ap()
    wp_t = nc.dram_tensor("wp", [HPC * HD, C], bf16, kind="ExternalInput").ap()
    w1_t = nc.dram_tensor("w1", [C, FPC], bf16, kind="ExternalInput").ap()
    b1_t = nc.dram_tensor("b1", [128, FT], f32, kind="ExternalInput").ap()
    w2_t = nc.dram_tensor("w2", [FPC, C], bf16, kind="ExternalInput").ap()
    out_t = nc.dram_tensor("outp", [C // TPG, T], bf16, kind="ExternalOutput").ap()

    xT_v = xT_t.rearrange("(kt p) t -> p kt t", p=128)
    wqk_v = wqk_t.rearrange("(kt p) f -> p kt f", p=128)
    wv_v = wv_t.rearrange("(kt p) f -> p kt f", p=128)
    wp_v = wp_t.rearrange("(kt p) c -> p kt c", p=128)
    w1_v = w1_t.rearrange("(kt p) f -> p kt f", p=128)
    w2_v = w2_t.rearrange("(ft p) c -> p ft c", p=128)

    with tile.TileContext(nc) as tc:
        with tc.tile_pool(name="cstp", bufs=1) as cst, \
             tc.tile_pool(name="ps", bufs=1, space="PSUM") as ps, \
             tc.tile_pool(name="dram", bufs=1, space="DRAM") as dram:

            masks = cst.tile([128, QC, TCH], f32, name="masks", tag="mask")
            nc.gpsimd.memset(masks[:], 0.0)
            for d in range(QC):
                nc.gpsimd.affine_select(
                    out=masks[:, d, :], in_=masks[:, d, :],
                    compare_op=mybir.AluOpType.is_ge,
                    fill=NEG, base=-d * 128,
                    pattern=[[1, TCH]], channel_multiplier=-1,
                )
            b1_sb = cst.tile([128, FT], f32, name="b1_sb", tag="b1")
            nc.sync.dma_start(b1_sb[:], b1_t)

            with tc.tile_pool(name="xp", bufs=1) as xp, \
                 tc.tile_pool(name="p1p", bufs=1) as p1p:
              xT_sb = xp.tile([128, KT, T], bf16, name="xT_sb", tag="xT")
              for c in range(QC):
                  nc.sync.dma_start(xT_sb[:, :, c * TCH:(c + 1) * TCH],
                                    xT_v[:, :, c * TCH:(c + 1) * TCH])
              for _rep in range(rep):
                with tc.tile_pool(name="attp", bufs=1) as attp, \
                     tc.tile_pool(name="ff1p", bufs=1) as ff1p:
                    attnT = attp.tile([128, HPC, T], bf16, name="attnT",
                                      tag="attnT")

                    # --- FF1 job-stream machinery -------------------------
                    ff1_jobs = [(c, f) for c in range(QC) for f in range(FT)]
                    ff1_dmas = {}
                    ff1_cur = [None]
                    ff1_ptr = [0]
                    h_tiles = {}
                    # During P2 the ScalarE stream is exp-heavy; emitting a
                    # Gelu there forces an ACT table-set reload (~2.7us) per
                    # exp<->gelu alternation (no set holds both). Defer: DVE
                    # copies the raw preact to SBUF, and one batched gelu
                    # pass runs at the P2->P3 boundary.
                    ff1_defer = [True]
                    ff1_deferred = []

                    def h_tile(c):
                        if c not in h_tiles:
                            h_tiles[c] = ff1p.tile([128, FT, TCH], bf16,
                                                   name="h_sb", tag="h",
                                                   bufs=2)
                        return h_tiles[c]

                    def ff1_w1dma(job):
                        if job >= len(ff1_jobs) or job in ff1_dmas:
                            return
                        _, f = ff1_jobs[job]
                        t = ff1p.tile([128, KT, 128], bf16, name="w1tile",
                                      tag="w1t", bufs=2)
                        nc.sync.dma_start(t[:],
                                          w1_v[:, :, f * 128:(f + 1) * 128])
                        ff1_dmas[job] = t

                    def ff1_emit(n_mms, limit_jobs):
                        # limit_jobs caps which jobs may be emitted: a chunk's
                        # jobs are only legal once the FF2 that frees its h
                        # buffer slot has been emitted (else the h WAR
                        # deadlocks the scalar FIFO).
                        for _ in range(n_mms):
                            idx = ff1_ptr[0]
                            if idx >= limit_jobs * KT:
                                return
                            job, k = divmod(idx, KT)
                            c, f = ff1_jobs[job]
                            if k == 0:
                                ff1_w1dma(job)
                                if job + 1 < limit_jobs:
                                    ff1_w1dma(job + 1)
                                ph = ps.tile([128, TCH], f32, name="ph",
                                             tag="ph", bufs=3)
                                ff1_cur[0] = (ff1_dmas.pop(job), ph)
                            w1tile, ph = ff1_cur[0]
                            nc.tensor.matmul(
                                ph[:], w1tile[:, k, :],
                                xT_sb[:, k, c * TCH:(c + 1) * TCH],
                                start=(k == 0), stop=(k == KT - 1))
                            if k == KT - 1:
                                if ff1_defer[0]:
                                    nc.vector.tensor_copy(
                                        h_tile(c)[:, f, :], ph[:])
                                    ff1_deferred.append((c, f))
                                else:
                                    nc.scalar.activation(
                                        h_tile(c)[:, f, :], ph[:],
                                        mybir.ActivationFunctionType.Gelu,
                                        bias=b1_sb[:, f:f + 1], scale=1.0)
                            ff1_ptr[0] = idx + 1

                    # ------------------------------------------------------

                    if not do_p12:
                        nc.gpsimd.memset(attnT[:], 0.01)
                    if do_p12:
                      with tc.tile_pool(name="qkvp", bufs=1) as qkvp:
                        qk_sb = qkvp.tile([128, 2 * HPC, T], bf16,
                                          name="qk_sb", tag="qk")
                        v_sb = qkvp.tile([128, T // 128, HPC * HD], bf16,
                                         name="v_sb", tag="v")

                        with tc.tile_pool(name="p1w", bufs=1) as p1w:
                            # P1a: qT/kT = w_qk^T @ x (feature-major);
                            # wqk streamed per 128-col tile from the
                            # persistent pool so the next rep's tiles
                            # prefetch during this rep's FF2
                            for ft in range(2 * HPC):
                                wqkt = p1p.tile([128, KT, 128], bf16,
                                                name="wqkt", tag="wqkt",
                                                bufs=3)
                                nc.sync.dma_start(
                                    wqkt[:],
                                    wqk_v[:, :, ft * 128:(ft + 1) * 128])
                                for c in range(QC):
                                    pt = ps.tile([128, TCH], f32, name="pmm",
                                                 tag="pmm", bufs=3)
                                    for k in range(KT):
                                        nc.tensor.matmul(
                                            pt[:],
                                            wqkt[:, k, :],
                                            xT_sb[:, k, c * TCH:(c + 1) * TCH],
                                            start=(k == 0), stop=(k == KT - 1))
                                    nc.vector.tensor_copy(
                                        qk_sb[:, ft, c * TCH:(c + 1) * TCH],
                                        pt[:])

                            # P1b: v = x @ w_v (token-major)
                            wv_sb = p1w.tile([128, KT, HPC * HD], bf16,
                                             name="wv_sb", tag="wv", bufs=1)
                            nc.sync.dma_start(wv_sb[:], wv_v)
                            for m in range(T // 128):
                                pt = ps.tile([128, HPC * HD], f32, name="pmm",
                                             tag="pmm", bufs=3)
                                for k in range(KT):
                                    nc.tensor.matmul(
                                        pt[:],
                                        xT_sb[:, k, m * 128:(m + 1) * 128],
                                        wv_sb[:, k, :],
                                        start=(k == 0), stop=(k == KT - 1))
                                nc.vector.tensor_copy(v_sb[:, m, :], pt[:])

                        # P2: causal attention with FF1(c0,c1) interleaved
                        # as PE filler between score(kt+1) and pv(kt).
                        with tc.tile_pool(name="p2w", bufs=1) as p2w:
                            LA = 1

                            def emit_score_exp(h, c, kt):
                                # diagonal block d contributes only to query
                                # cols >= d*128
                                off = max(0, (kt - 4 * c) * 128)
                                pscore = ps.tile([128, TCH], f32, name="pmm",
                                                 tag="pmm", bufs=3)
                                nc.tensor.matmul(
                                    pscore[:, off:],
                                    qk_sb[:, HPC + h,
                                          kt * 128:(kt + 1) * 128],
                                    qk_sb[:, h,
                                          c * TCH + off:(c + 1) * TCH],
                                    start=True, stop=True)
                                e_sb = p2w.tile([128, TCH], bf16, name="e_sb",
                                                tag="e", bufs=3)
                                if kt >= 4 * c:
                                    d = kt - 4 * c
                                    ms = p2w.tile([128, TCH], f32, name="ms",
                                                  tag="ms", bufs=2)
                                    nc.vector.tensor_add(
                                        ms[:, off:], pscore[:, off:],
                                        masks[:, d, off:])
                                    nc.scalar.activation(
                                        e_sb[:, off:], ms[:, off:],
                                        mybir.ActivationFunctionType.Exp,
                                        scale=SM_SCALE)
                                else:
                                    nc.scalar.activation(
                                        e_sb[:], pscore[:],
                                        mybir.ActivationFunctionType.Exp,
                                        scale=SM_SCALE)
                                return e_sb, off

                            def emit_norm(st):
                                po, e_sum, h, c = st
                                rb = p2w.tile([128, TCH], f32, name="rb",
                                              tag="rb", bufs=2)
                                nc.vector.reciprocal(rb[:], e_sum[:])
                                nc.vector.tensor_mul(
                                    attnT[:, h, c * TCH:(c + 1) * TCH],
                                    po[:], rb[:])

                            pending = None
                            for h in range(HPC):
                                for c in range(QC):
                                    nkt = 4 * c + 4
                                    po = ps.tile([128, TCH], f32, name="po",
                                                 tag="po", bufs=2)
                                    e_acc = p2w.tile([128, TCH], f32,
                                                     name="e_acc", tag="eacc",
                                                     bufs=2)
                                    es = {}
                                    for kt in range(min(LA, nkt)):
                                        es[kt] = emit_score_exp(h, c, kt)
                                    for kt in range(nkt):
                                        if kt + LA < nkt:
                                            es[kt + LA] = emit_score_exp(
                                                h, c, kt + LA)
                                        ff1_emit(3, 2 * FT)
                                        e_sb, off = es.pop(kt)
                                        if kt == 0:
                                            nc.vector.tensor_copy(
                                                e_acc[:], e_sb[:])
                                        else:
                                            nc.vector.tensor_add(
                                                e_acc[:, off:],
                                                e_acc[:, off:],
                                                e_sb[:, off:])
                                        nc.tensor.matmul(
                                            po[:, off:],
                                            v_sb[:, kt, h * HD:(h + 1) * HD],
                                            e_sb[:, off:], start=(kt == 0),
                                            stop=(kt == nkt - 1))
                                    e_sum = p2w.tile([128, TCH], f32,
                                                     name="e_sum", tag="esum",
                                                     bufs=2)
                                    nc.gpsimd.partition_all_reduce(
                                        e_sum[:], e_acc[:], channels=128,
                                        reduce_op=bass_isa.ReduceOp.add)
                                    if pending is not None:
                                        # deferred a full group: the gpsimd
                                        # reduce gets a group-span of slack
                                        # before the DVE waits on it
                                        emit_norm(pending)
                                    pending = (po, e_sum, h, c)
                            emit_norm(pending)

                    if not do_p3:
                        nc.sync.dma_start(out_t[0:128, :], attnT[:, 0, :])
                    if do_p3:
                      with tc.tile_pool(name="p3w", bufs=1) as p3w:
                        # Batched gelu for P2-deferred FF1 tiles (in place),
                        # chunk 0 first so FF2(c0) unblocks ASAP; exactly one
                        # gelu table load per rep. Leftover FF1 jobs below
                        # then gelu directly (set already resident).
                        ff1_defer[0] = False
                        for (c, f) in ff1_deferred:
                            nc.scalar.activation(
                                h_tiles[c][:, f, :], h_tiles[c][:, f, :],
                                mybir.ActivationFunctionType.Gelu,
                                bias=b1_sb[:, f:f + 1], scale=1.0)
                        del ff1_deferred[:]
                        ff1_emit(10 ** 9, 2 * FT)   # leftovers of chunks 0-1
                        wp_sb = p3w.tile([128, TPG, C], bf16, name="wp_sb",
                                         tag="wp", bufs=1)
                        nc.sync.dma_start(wp_sb[:], wp_v)
                        rs_out_all = dram.tile([QC, (COT * 128) // TPG, TCH],
                                               bf16, name="rs_out_all",
                                               tag="rsoa", bufs=2)

                        def emit_ff2(c, prefetch_jobs=0):
                            ht = h_tiles[c]
                            rs_in = dram.tile([COT * 128, TCH], bf16,
                                              name="rs_in", tag="rsi", bufs=2)
                            for co in range(COT):
                                if co == COT - 2 and prefetch_jobs:
                                    # warm the next FF1 chunk's first weight
                                    # tiles so its matmuls start without a
                                    # DMA cold-stall (PE p-state ramp)
                                    j = ff1_ptr[0] // KT
                                    ff1_w1dma(j)
                                    ff1_w1dma(j + 1)
                                w2tile = p3w.tile([128, FT, 128], bf16,
                                                  name="w2tile", tag="w2t",
                                                  bufs=3)
                                nc.sync.dma_start(
                                    w2tile[:],
                                    w2_v[:, :, co * 128:(co + 1) * 128])
                                pout = ps.tile([128, TCH], f32, name="pmm",
                                               tag="pmm", bufs=3)
                                # ff2 first, proj last: the first groups can
                                # start before wp finishes loading
                                for f in range(FT):
                                    nc.tensor.matmul(
                                        pout[:], w2tile[:, f, :], ht[:, f, :],
                                        start=(f == 0), stop=False)
                                for k4 in range(TPG):
                                    nc.tensor.matmul(
                                        pout[:],
                                        wp_sb[:, k4, co * 128:(co + 1) * 128],
                                        attnT[:, k4, c * TCH:(c + 1) * TCH],
                                        start=False, stop=(k4 == TPG - 1))
                                o_sb = p3w.tile([128, TCH], bf16, name="o_sb",
                                                tag="o", bufs=2)
                                nc.vector.tensor_copy(o_sb[:], pout[:])
                                # staging store on the scalar HWDGE queue
                                nc.scalar.dma_start(
                                    rs_in[co * 128:(co + 1) * 128, :], o_sb[:])
                            if rs_mode == "chunked":
                                nc.gpsimd.collective_compute(
                                    "ReduceScatter", mybir.AluOpType.add,
                                    replica_groups=[[0, 1, 2, 3], [4, 5, 6, 7]],
                                    ins=[rs_in.opt()], outs=[rs_out_all[c]])
                            elif rs_mode == "none":
                                nc.sync.dma_start(
                                    out_t[:, c * TCH:(c + 1) * TCH],
                                    rs_in[0:512, :])

                        if 0 not in h_tiles:        # do_p12=False ablation
                            ff1_emit(10 ** 9, 2 * FT)
                        emit_ff2(0, prefetch_jobs=1)
                        ff1_emit(10 ** 9, 3 * FT)   # chunk 2
                        emit_ff2(1, prefetch_jobs=1)
                        ff1_emit(10 ** 9, 4 * FT)   # chunk 3
                        emit_ff2(2)
                        emit_ff2(3)
                        if rs_mode == "chunked":
                            # end-of-rep out copies on the gpsimd queue
                            # (after all RS triggers)
                            for c in range(QC):
                                nc.gpsimd.dma_start(
                                    out_t[:, c * TCH:(c + 1) * TCH],
                                    rs_out_all[c])

    nc.compile()
    return nc


def make_in_maps(x, w_qkv, w_proj, w_ff1, b_ff1, w_ff2):
    in_maps = []
    asc = np.ascontiguousarray
    bf = ml_dtypes.bfloat16
    for r in range(NCORES):
        b, hg = r // TPG, r % TPG
        q_cols = w_qkv[:, hg * 512:(hg + 1) * 512]
        k_cols = w_qkv[:, C + hg * 512:C + (hg + 1) * 512]
        v_cols = w_qkv[:, 2 * C + hg * 512:2 * C + (hg + 1) * 512]
        xT = asc(x[b].T)
        in_maps.append({
            "xT": xT.astype(bf),
            "wqk": asc(np.concatenate([q_cols, k_cols], axis=1)).astype(bf),
            "wv": asc(v_cols).astype(bf),
            "wp": asc(w_proj[hg * 512:(hg + 1) * 512, :]).astype(bf),
            "w1": asc(w_ff1[:, hg * FPC:(hg + 1) * FPC]).astype(bf),
            "b1": asc(b_ff1[hg * FPC:(hg + 1) * FPC].reshape(FT, 128).T),
            "w2": asc(w_ff2[hg * FPC:(hg + 1) * FPC, :]).astype(bf),
        })
    return in_maps


def assemble(results, x, b_ff2):
    out = np.empty((B, T, C), np.float32)
    for r in range(NCORES):
        b, idx = r // TPG, r % TPG
        out[b, :, idx * 512:(idx + 1) * 512] = \
            results[r]["outp"].astype(np.float32).T
    out += x + b_ff2
    return out


def kernel(x, w_qkv, w_proj, w_ff1, b_ff1, w_ff2, b_ff2):
    global _CACHED_NC
    x = np.asarray(x, np.float32)
    if _CACHED_NC is None:
        _CACHED_NC = build_nc()
    in_maps = make_in_maps(x, np.asarray(w_qkv, np.float32),
                           np.asarray(w_proj, np.float32),
                           np.asarray(w_ff1, np.float32),
                           np.asarray(b_ff1, np.float32),
                           np.asarray(w_ff2, np.float32))
    res = bass_utils.run_bass_kernel_spmd(_CACHED_NC, in_maps,
                                          core_ids=list(range(NCORES)))
    return assemble(res.results, x, np.asarray(b_ff2, np.float32))



# revision 7
# speedup vs baseline: 1.1265x; 1.0873x over previous
"""Transformer block (QKV + causal MHA + proj + GELU-FF, residual) on 8 NeuronCores.

Sharding: DP over batch (2 groups of 4 cores) x TP over heads / FF-inner within
each group. Identical SPMD program on all cores; per-core differences are input
slices only. Activations are feature-major end to end; all matmul operands bf16
(PSUM accumulation fp32). xT is loaded once and kept resident in SBUF for P1a
(QK), P1b (V) and FF1. FF1 for token chunks 0-1 is interleaved into the
attention kt loop as PE filler (the PE queue is strict FIFO; filler matmuls
cover the ScalarE exp latency between score and pv matmuls); FF1 for chunks
2-3 runs after FF2 of chunks 0-1 frees their h buffers (emitting them earlier
would deadlock the PE FIFO on the h WAR). exp tiles accumulate on the DVE; one
ones-matmul per (h,c) group forms the softmax denominators. proj and ff2
partials share a PSUM group; token-chunked bf16 ReduceScatter per 4-core
group, with end-of-rep output copies on the gpsimd queue. Host adds x + b_ff2
(residual) during unshard.
"""
import numpy as np
import ml_dtypes

import concourse.bass as bass
import concourse.bass_isa as bass_isa
import concourse.mybir as mybir
import concourse.tile as tile
from concourse import bacc
from concourse import bass_utils

B, T, C = 2, 2048, 2048
H, HD = 16, 128
F = 8192
NCORES = 8
TPG = 4                  # cores per batch group
HPC = H // TPG           # heads per core
QC = 4                   # token chunks per batch
TCH = T // QC            # 512
KT = C // 128            # 16
FPC = F // TPG           # 2048 ff rows per core
FT = FPC // 128          # 16
COT = C // 128           # 16
SM_SCALE = 1.0 / float(np.sqrt(HD))
NEG = -60000.0

f32r = mybir.dt.float32r
f32 = mybir.dt.float32
bf16 = mybir.dt.bfloat16

_CACHED_NC = None


def build_nc(rep=1, do_p12=True, do_p3=True, rs_mode="chunked"):
    nc = bacc.Bacc("TRN2", target_bir_lowering=False, debug=False,
                   num_devices=NCORES)
    xT_t = nc.dram_tensor("xT", [C, T], bf16, kind="ExternalInput").ap()
    wqk_t = nc.dram_tensor("wqk", [C, 2 * HPC * HD], bf16, kind="ExternalInput").ap()
    wv_t = nc.dram_tensor("wv", [C, HPC * HD], bf16, kind="ExternalInput").ap()
    wp_t = nc.dram_tensor("wp", [HPC * HD, C], bf16, kind="ExternalInput").ap()
    w1_t = nc.dram_tensor("w1", [C, FPC], bf16, kind="ExternalInput").ap()
    b1_t = nc.dram_tensor("b1", [128, FT], f32, kind="ExternalInput").ap()
    w2_t = nc.dram_tensor("w2", [FPC, C], bf16, kind="ExternalInput").ap()
    out_t = nc.dram_tensor("outp", [C // TPG, T], bf16, kind="ExternalOutput").ap()

    xT_v = xT_t.rearrange("(kt p) t -> p kt t", p=128)
    wqk_v = wqk_t.rearrange("(kt p) f -> p kt f", p=128)
    wv_v = wv_t.rearrange("(kt p) f -> p kt f", p=128)
    wp_v = wp_t.rearrange("(kt p) c -> p kt c", p=128)
    w1_v = w1_t.rearrange("(kt p) f -> p kt f", p=128)
    w2_v = w2_t.rearrange("(ft p) c -> p ft c", p=128)

    with tile.TileContext(nc) as tc:
        with tc.tile_pool(name="cstp", bufs=1) as cst, \
             tc.tile_pool(name="ps", bufs=1, space="PSUM") as ps, \
             tc.tile_pool(name="dram", bufs=1, space="DRAM") as dram:

            masks = cst.tile([128, QC, TCH], f32, name="masks", tag="mask")
            nc.gpsimd.memset(masks[:], 0.0)
            for d in range(QC):
                nc.gpsimd.affine_select(
                    out=masks[:, d, :], in_=masks[:, d, :],
                    compare_op=mybir.AluOpType.is_ge,
                    fill=NEG, base=-d * 128,
                    pattern=[[1, TCH]], channel_multiplier=-1,
                )
            b1_sb = cst.tile([128, FT], f32, name="b1_sb", tag="b1")
            nc.sync.dma_start(b1_sb[:], b1_t)

            with tc.tile_pool(name="xp", bufs=1) as xp, \
                 tc.tile_pool(name="p1p", bufs=1) as p1p:
              xT_sb = xp.tile([128, KT, T], bf16, name="xT_sb", tag="xT")
              for c in range(QC):
                  nc.sync.dma_start(xT_sb[:, :, c * TCH:(c + 1) * TCH],
                                    xT_v[:, :, c * TCH:(c + 1) * TCH])
              for _rep in range(rep):
                with tc.tile_pool(name="attp", bufs=1) as attp, \
                     tc.tile_pool(name="ff1p", bufs=1) as ff1p:
                    attnT = attp.tile([128, HPC, T], bf16, name="attnT",
                                      tag="attnT")

                    # --- FF1 job-stream machinery -------------------------
                    ff1_jobs = [(c, f) for c in range(QC) for f in range(FT)]
                    ff1_dmas = {}
                    ff1_cur = [None]
                    ff1_ptr = [0]
                    h_tiles = {}
                    # During P2 the ScalarE stream is exp-heavy; emitting a
                    # Gelu there forces an ACT table-set reload (~2.7us) per
                    # exp<->gelu alternation (no set holds both). Defer: DVE
                    # copies the raw preact to SBUF, and one batched gelu
                    # pass runs at the P2->P3 boundary.
                    ff1_defer = [True]
                    ff1_deferred = []

                    def h_tile(c):
                        if c not in h_tiles:
                            h_tiles[c] = ff1p.tile([128, FT, TCH], bf16,
                                                   name="h_sb", tag="h",
                                                   bufs=2)
                        return h_tiles[c]

                    def ff1_w1dma(job):
                        if job >= len(ff1_jobs) or job in ff1_dmas:
                            return
                        _, f = ff1_jobs[job]
                        t = ff1p.tile([128, KT, 128], bf16, name="w1tile",
                                      tag="w1t", bufs=2)
                        nc.sync.dma_start(t[:],
                                          w1_v[:, :, f * 128:(f + 1) * 128])
                        ff1_dmas[job] = t

                    def ff1_emit(n_mms, limit_jobs):
                        # limit_jobs caps which jobs may be emitted: a chunk's
                        # jobs are only legal once the FF2 that frees its h
                        # buffer slot has been emitted (else the h WAR
                        # deadlocks the scalar FIFO).
                        for _ in range(n_mms):
                            idx = ff1_ptr[0]
                            if idx >= limit_jobs * KT:
                                return
                            job, k = divmod(idx, KT)
                            c, f = ff1_jobs[job]
                            if k == 0:
                                ff1_w1dma(job)
                                if job + 1 < limit_jobs:
                                    ff1_w1dma(job + 1)
                                ph = ps.tile([128, TCH], f32, name="ph",
                                             tag="ph", bufs=3)
                                ff1_cur[0] = (ff1_dmas.pop(job), ph)
                            w1tile, ph = ff1_cur[0]
                            nc.tensor.matmul(
                                ph[:], w1tile[:, k, :],
                                xT_sb[:, k, c * TCH:(c + 1) * TCH],
                                start=(k == 0), stop=(k == KT - 1))
                            if k == KT - 1:
                                if ff1_defer[0]:
                                    nc.vector.tensor_copy(
                                        h_tile(c)[:, f, :], ph[:])
                                    ff1_deferred.append((c, f))
                                else:
                                    nc.scalar.activation(
                                        h_tile(c)[:, f, :], ph[:],
                                        mybir.ActivationFunctionType.Gelu,
                                        bias=b1_sb[:, f:f + 1], scale=1.0)
                            ff1_ptr[0] = idx + 1

                    # ------------------------------------------------------

                    if not do_p12:
                        nc.gpsimd.memset(attnT[:], 0.01)
                    if do_p12:
                      with tc.tile_pool(name="qkvp", bufs=1) as qkvp:
                        qk_sb = qkvp.tile([128, 2 * HPC, T], bf16,
                                          name="qk_sb", tag="qk")
                        v_sb = qkvp.tile([128, T // 128, HPC * HD], bf16,
                                         name="v_sb", tag="v")

                        with tc.tile_pool(name="p1w", bufs=1) as p1w:
                            # P1a: qT/kT = w_qk^T @ x (feature-major);
                            # wqk streamed per 128-col tile from the
                            # persistent pool so the next rep's tiles
                            # prefetch during this rep's FF2
                            for ft in range(2 * HPC):
                                wqkt = p1p.tile([128, KT, 128], bf16,
                                                name="wqkt", tag="wqkt",
                                                bufs=3)
                                nc.sync.dma_start(
                                    wqkt[:],
                                    wqk_v[:, :, ft * 128:(ft + 1) * 128])
                                for c in range(QC):
                                    pt = ps.tile([128, TCH], f32, name="pmm",
                                                 tag="pmm", bufs=3)
                                    for k in range(KT):
                                        nc.tensor.matmul(
                                            pt[:],
                                            wqkt[:, k, :],
                                            xT_sb[:, k, c * TCH:(c + 1) * TCH],
                                            start=(k == 0), stop=(k == KT - 1))
                                    nc.vector.tensor_copy(
                                        qk_sb[:, ft, c * TCH:(c + 1) * TCH],
                                        pt[:])

                            # P1b: v = x @ w_v (token-major)
                            wv_sb = p1w.tile([128, KT, HPC * HD], bf16,
                                             name="wv_sb", tag="wv", bufs=1)
                            nc.sync.dma_start(wv_sb[:], wv_v)
                            for m in range(T // 128):
                                pt = ps.tile([128, HPC * HD], f32, name="pmm",
                                             tag="pmm", bufs=3)
                                for k in range(KT):
                                    nc.tensor.matmul(
                                        pt[:],
                                        xT_sb[:, k, m * 128:(m + 1) * 128],
                                        wv_sb[:, k, :],
                                        start=(k == 0), stop=(k == KT - 1))
                                nc.vector.tensor_copy(v_sb[:, m, :], pt[:])

                        # P2: causal attention with FF1(c0,c1) interleaved
                        # as PE filler between score(kt+1) and pv(kt).
                        with tc.tile_pool(name="p2w", bufs=1) as p2w:
                            LA = 1

                            def emit_score_exp(h, c, kt):
                                # diagonal block d contributes only to query
                                # cols >= d*128
                                off = max(0, (kt - 4 * c) * 128)
                                pscore = ps.tile([128, TCH], f32, name="pmm",
                                                 tag="pmm", bufs=3)
                                nc.tensor.matmul(
                                    pscore[:, off:],
                                    qk_sb[:, HPC + h,
                                          kt * 128:(kt + 1) * 128],
                                    qk_sb[:, h,
                                          c * TCH + off:(c + 1) * TCH],
                                    start=True, stop=True)
                                e_sb = p2w.tile([128, TCH], bf16, name="e_sb",
                                                tag="e", bufs=3)
                                if kt >= 4 * c:
                                    d = kt - 4 * c
                                    ms = p2w.tile([128, TCH], f32, name="ms",
                                                  tag="ms", bufs=2)
                                    nc.vector.tensor_add(
                                        ms[:, off:], pscore[:, off:],
                                        masks[:, d, off:])
                                    nc.scalar.activation(
                                        e_sb[:, off:], ms[:, off:],
                                        mybir.ActivationFunctionType.Exp,
                                        scale=SM_SCALE)
                                else:
                                    nc.scalar.activation(
                                        e_sb[:], pscore[:],
                                        mybir.ActivationFunctionType.Exp,
                                        scale=SM_SCALE)
                                return e_sb, off

                            def emit_norm(st):
                                po, e_sum, h, c = st
                                rb = p2w.tile([128, TCH], f32, name="rb",
                                              tag="rb", bufs=2)
                                nc.vector.reciprocal(rb[:], e_sum[:])
                                nc.vector.tensor_mul(
                                    attnT[:, h, c * TCH:(c + 1) * TCH],
                                    po[:], rb[:])

                            pending = None
                            last_esum = [None]
                            for h in range(HPC):
                                for c in range(QC):
                                    nkt = 4 * c + 4
                                    po = ps.tile([128, TCH], f32, name="po",
                                                 tag="po", bufs=2)
                                    e_acc = p2w.tile([128, TCH], f32,
                                                     name="e_acc", tag="eacc",
                                                     bufs=2)
                                    es = {}
                                    for kt in range(min(LA, nkt)):
                                        es[kt] = emit_score_exp(h, c, kt)
                                    for kt in range(nkt):
                                        if kt + LA < nkt:
                                            es[kt + LA] = emit_score_exp(
                                                h, c, kt + LA)
                                        ff1_emit(3, 2 * FT)
                                        e_sb, off = es.pop(kt)
                                        if kt == 0:
                                            nc.vector.tensor_copy(
                                                e_acc[:], e_sb[:])
                                        else:
                                            nc.vector.tensor_add(
                                                e_acc[:, off:],
                                                e_acc[:, off:],
                                                e_sb[:, off:])
                                        nc.tensor.matmul(
                                            po[:, off:],
                                            v_sb[:, kt, h * HD:(h + 1) * HD],
                                            e_sb[:, off:], start=(kt == 0),
                                            stop=(kt == nkt - 1))
                                    e_sum = p2w.tile([128, TCH], f32,
                                                     name="e_sum", tag="esum",
                                                     bufs=2)
                                    nc.gpsimd.partition_all_reduce(
                                        e_sum[:], e_acc[:], channels=128,
                                        reduce_op=bass_isa.ReduceOp.add)
                                    if pending is not None:
                                        # deferred a full group: the gpsimd
                                        # reduce gets a group-span of slack
                                        # before the DVE waits on it
                                        emit_norm(pending)
                                    pending = (po, e_sum, h, c)
                                    last_esum[0] = e_sum
                            emit_norm(pending)

                    if not do_p3:
                        nc.sync.dma_start(out_t[0:128, :], attnT[:, 0, :])
                    if do_p3:
                      with tc.tile_pool(name="p3w", bufs=1) as p3w:
                        ff1_emit(10 ** 9, 2 * FT)   # leftovers of chunks 0-1
                        # Batched gelu for the deferred FF1 tiles (in place),
                        # chunk 0 first so FF2(c0) unblocks ASAP; exactly one
                        # gelu table load per rep. The Tile scheduler would
                        # otherwise hoist these into the P2 exp stream (their
                        # raw tiles finish mid-P2) re-thrashing the ACT
                        # tables, so pin them behind P2 with a bias tile that
                        # data-depends on the last softmax denominator.
                        ff1_defer[0] = False
                        if last_esum[0] is not None:
                            zg = p3w.tile([128, 1], f32, name="zgate",
                                          tag="zg", bufs=1)
                            nc.vector.tensor_scalar_mul(
                                zg[:], last_esum[0][:, 0:1], 0.0)
                            b1_gelu = p3w.tile([128, FT], f32,
                                               name="b1_late", tag="b1l",
                                               bufs=1)
                            nc.vector.tensor_scalar_add(
                                b1_gelu[:], b1_sb[:], zg[:])
                        else:
                            b1_gelu = b1_sb
                        for (c, f) in ff1_deferred:
                            nc.scalar.activation(
                                h_tiles[c][:, f, :], h_tiles[c][:, f, :],
                                mybir.ActivationFunctionType.Gelu,
                                bias=b1_gelu[:, f:f + 1], scale=1.0)
                        del ff1_deferred[:]
                        wp_sb = p3w.tile([128, TPG, C], bf16, name="wp_sb",
                                         tag="wp", bufs=1)
                        nc.sync.dma_start(wp_sb[:], wp_v)
                        rs_out_all = dram.tile([QC, (COT * 128) // TPG, TCH],
                                               bf16, name="rs_out_all",
                                               tag="rsoa", bufs=2)

                        def emit_ff2(c, prefetch_jobs=0):
                            ht = h_tiles[c]
                            rs_in = dram.tile([COT * 128, TCH], bf16,
                                              name="rs_in", tag="rsi", bufs=2)
                            for co in range(COT):
                                if co == COT - 2 and prefetch_jobs:
                                    # warm the next FF1 chunk's first weight
                                    # tiles so its matmuls start without a
                                    # DMA cold-stall (PE p-state ramp)
                                    j = ff1_ptr[0] // KT
                                    ff1_w1dma(j)
                                    ff1_w1dma(j + 1)
                                w2tile = p3w.tile([128, FT, 128], bf16,
                                                  name="w2tile", tag="w2t",
                                                  bufs=3)
                                nc.sync.dma_start(
                                    w2tile[:],
                                    w2_v[:, :, co * 128:(co + 1) * 128])
                                pout = ps.tile([128, TCH], f32, name="pmm",
                                               tag="pmm", bufs=3)
                                # ff2 first, proj last: the first groups can
                                # start before wp finishes loading
                                for f in range(FT):
                                    nc.tensor.matmul(
                                        pout[:], w2tile[:, f, :], ht[:, f, :],
                                        start=(f == 0), stop=False)
                                for k4 in range(TPG):
                                    nc.tensor.matmul(
                                        pout[:],
                                        wp_sb[:, k4, co * 128:(co + 1) * 128],
                                        attnT[:, k4, c * TCH:(c + 1) * TCH],
                                        start=False, stop=(k4 == TPG - 1))
                                o_sb = p3w.tile([128, TCH], bf16, name="o_sb",
                                                tag="o", bufs=2)
                                nc.vector.tensor_copy(o_sb[:], pout[:])
                                # staging store on the scalar HWDGE queue
                                nc.scalar.dma_start(
                                    rs_in[co * 128:(co + 1) * 128, :], o_sb[:])
                            if rs_mode == "chunked":
                                nc.gpsimd.collective_compute(
                                    "ReduceScatter", mybir.AluOpType.add,
                                    replica_groups=[[0, 1, 2, 3], [4, 5, 6, 7]],
                                    ins=[rs_in.opt()], outs=[rs_out_all[c]])
                            elif rs_mode == "none":
                                nc.sync.dma_start(
                                    out_t[:, c * TCH:(c + 1) * TCH],
                                    rs_in[0:512, :])

                        if 0 not in h_tiles:        # do_p12=False ablation
                            ff1_emit(10 ** 9, 2 * FT)
                        emit_ff2(0, prefetch_jobs=1)
                        ff1_emit(10 ** 9, 3 * FT)   # chunk 2
                        emit_ff2(1, prefetch_jobs=1)
                        ff1_emit(10 ** 9, 4 * FT)   # chunk 3
                        emit_ff2(2)
                        emit_ff2(3)
                        if rs_mode == "chunked":
                            # end-of-rep out copies on the gpsimd queue
                            # (after all RS triggers)
                            for c in range(QC):
                                nc.gpsimd.dma_start(
                                    out_t[:, c * TCH:(c + 1) * TCH],
                                    rs_out_all[c])

    nc.compile()
    return nc


def make_in_maps(x, w_qkv, w_proj, w_ff1, b_ff1, w_ff2):
    in_maps = []
    asc = np.ascontiguousarray
    bf = ml_dtypes.bfloat16
    for r in range(NCORES):
        b, hg = r // TPG, r % TPG
        q_cols = w_qkv[:, hg * 512:(hg + 1) * 512]
        k_cols = w_qkv[:, C + hg * 512:C + (hg + 1) * 512]
        v_cols = w_qkv[:, 2 * C + hg * 512:2 * C + (hg + 1) * 512]
        xT = asc(x[b].T)
        in_maps.append({
            "xT": xT.astype(bf),
            "wqk": asc(np.concatenate([q_cols, k_cols], axis=1)).astype(bf),
            "wv": asc(v_cols).astype(bf),
            "wp": asc(w_proj[hg * 512:(hg + 1) * 512, :]).astype(bf),
            "w1": asc(w_ff1[:, hg * FPC:(hg + 1) * FPC]).astype(bf),
            "b1": asc(b_ff1[hg * FPC:(hg + 1) * FPC].reshape(FT, 128).T),
            "w2": asc(w_ff2[hg * FPC:(hg + 1) * FPC, :]).astype(bf),
        })
    return in_maps


def assemble(results, x, b_ff2):
    out = np.empty((B, T, C), np.float32)
    for r in range(NCORES):
        b, idx = r // TPG, r % TPG
        out[b, :, idx * 512:(idx + 1) * 512] = \
            results[r]["outp"].astype(np.float32).T
    out += x + b_ff2
    return out


def kernel(x, w_qkv, w_proj, w_ff1, b_ff1, w_ff2, b_ff2):
    global _CACHED_NC
    x = np.asarray(x, np.float32)
    if _CACHED_NC is None:
        _CACHED_NC = build_nc()
    in_maps = make_in_maps(x, np.asarray(w_qkv, np.float32),
                           np.asarray(w_proj, np.float32),
                           np.asarray(w_ff1, np.float32),
                           np.asarray(b_ff1, np.float32),
                           np.asarray(w_ff2, np.float32))
    res = bass_utils.run_bass_kernel_spmd(_CACHED_NC, in_maps,
                                          core_ids=list(range(NCORES)))
    return assemble(res.results, x, np.asarray(b_ff2, np.float32))

